# revision 15
# baseline (speedup 1.0000x reference)
"""Trainium2 Bass kernel for speaker-rate positional-encoding attention.

Math (per batch b):
  rate_q = sigmoid(spk @ Wsq.T + bsq);  rate_k = sigmoid(spk @ Wsk.T + bsk)
  pe(x)[l,d] = sin(rate * pos[l] * 10000^(-d/D) + phase[d]),  phase = 0/pi/2 (even/odd d)
  q = (query + pe_q) @ Wq.T + bq ; k = (keys + pe_k) @ Wk.T + bk
  v1 = values @ Wv.T + bv ; vpp = v1 @ Wo.T
  E = exp((k q^T)/sqrt(D)) ; out[t] = (sum_s E[s,t] vpp[s,:]) / (sqrt(D)*sum_s E[s,t]) + bo

Implementation notes:
 - 8 NeuronCores, data-parallel over batch (2 per core), no collectives.
 - All activations live feature-on-partition ("transposed"); the host
   pre-transposes inputs and weights so the device never transposes.
 - Matmuls run in float32r (full-rate fp32 on the PE for N>=256).
 - Softmax skips max-subtraction (scores bounded ~ +-15) and defers
   normalization to a per-partition scalar multiply after E @ vpp.
 - Sin LUT domain is [-pi, pi]: angles are range-reduced with
   ((x + pi) mod 2pi) - pi  (all angles >= 0 so floor-mod is safe).
"""

import sys

for _p in ("/opt/trn_rl_repo",):
    if _p not in sys.path:
        sys.path.insert(0, _p)

import numpy as np

import concourse.bass as bass
from concourse import bacc
import concourse.mybir as mybir
import concourse.tile as tile
from concourse.bass_utils import run_bass_kernel_spmd

B, T, S, D, SPK = 16, 1024, 512, 1024, 256
NCORES = 8
BL = B // NCORES          # batches per core
P = 128
DT = D // P               # 8 d-tiles (also j-tiles / e-tiles)
ST = S // P               # 4 s-tiles
TT = T // P               # 8 t-tiles
F32 = mybir.dt.float32
MMDT = mybir.dt.float32r  # matmul compute dtype
SQRT_D = float(np.sqrt(D))
PI = float(np.pi)

AF = mybir.ActivationFunctionType
ALU = mybir.AluOpType


def build_nc():
    nc = bacc.Bacc()
    dp = nc.declare_dram_parameter
    qTi = dp("qTi", [BL, DT, P, T], MMDT, isOutput=False)   # query^T  [b,dt,p_d,t]
    kTi = dp("kTi", [BL, DT, P, S], MMDT, isOutput=False)   # keys^T
    vTi = dp("vTi", [BL, DT, P, S], MMDT, isOutput=False)   # values^T
    wq = dp("wq", [DT, P, D], MMDT, isOutput=False)         # Wq.T [dt,p_d,j]
    wk = dp("wk", [DT, P, D], MMDT, isOutput=False)
    wv = dp("wv", [DT, P, D], MMDT, isOutput=False)
    wo = dp("wo", [DT, P, D], MMDT, isOutput=False)         # Wo.T [jt,p_j,e]
    posq = dp("posq", [T], F32, isOutput=False)             # current_mel_pos + t
    posk = dp("posk", [S], F32, isOutput=False)
    invd = dp("invd", [P, DT], F32, isOutput=False)         # 10000^(-d/D)
    phs = dp("phs", [P, DT], F32, isOutput=False)           # 0 / pi/2
    bqc = dp("bqc", [P, DT], F32, isOutput=False)
    bkc = dp("bkc", [P, DT], F32, isOutput=False)
    bvc = dp("bvc", [P, DT], F32, isOutput=False)
    boc = dp("boc", [D], F32, isOutput=False)
    spk = dp("spk", [P, 2 * BL], F32, isOutput=False)      # spk^T  [p, kt*BL+b]
    wsq = dp("wsq", [P, 2 * P], F32, isOutput=False)       # Wsq col-replicated
    wsk = dp("wsk", [P, 2 * P], F32, isOutput=False)
    bsq = dp("bsq", [P, 1], F32, isOutput=False)
    sumw = dp("sumw", [P, 2], MMDT, isOutput=False)   # sqrt(D) ones column
    bsk = dp("bsk", [P, 1], F32, isOutput=False)
    out = dp("out", [BL, TT, P, D], F32, isOutput=True)

    def bcast(ap, n=P):
        # replicate a DRAM vector across n partitions (DMA partition-step 0)
        return bass.AP(tensor=ap.tensor, offset=ap.offset, ap=[[0, n], *ap.ap])

    with tile.TileContext(nc) as tc:
        with (
            tc.tile_pool(name="consts", bufs=1) as cp,
            tc.tile_pool(name="wch", bufs=8) as wp,       # weight chunks [P, D]
            tc.tile_pool(name="inch", bufs=6) as ip,      # input act chunks
            tc.tile_pool(name="tmp", bufs=6) as tp,       # posenc temps
            tc.tile_pool(name="big", bufs=1) as bp,       # long-lived per-batch acts
            tc.tile_pool(name="outp", bufs=4) as op,
            tc.tile_pool(name="psum", bufs=8, space="PSUM") as pp,
        ):
            # ---------------- constants ----------------
            posq_bc = cp.tile([P, T], F32)
            nc.gpsimd.dma_start(out=posq_bc, in_=bcast(posq[:]))
            posk_bc = cp.tile([P, S], F32)
            nc.gpsimd.dma_start(out=posk_bc, in_=bcast(posk[:]))
            bo_bc = cp.tile([P, D], F32)
            nc.gpsimd.dma_start(out=bo_bc, in_=bcast(boc[:]))
            invd_sb = cp.tile([P, DT], F32)
            nc.sync.dma_start(out=invd_sb, in_=invd[:])
            phs_sb = cp.tile([P, DT], F32)
            nc.sync.dma_start(out=phs_sb, in_=phs[:])
            bq_sb = cp.tile([P, DT], F32)
            nc.sync.dma_start(out=bq_sb, in_=bqc[:])
            bk_sb = cp.tile([P, DT], F32)
            nc.sync.dma_start(out=bk_sb, in_=bkc[:])
            bv_sb = cp.tile([P, DT], F32)
            nc.sync.dma_start(out=bv_sb, in_=bvc[:])
            spk_sb = cp.tile([P, 2 * BL], F32)
            nc.sync.dma_start(out=spk_sb, in_=spk[:])
            wsq_sb = cp.tile([P, 2 * P], F32)
            nc.sync.dma_start(out=wsq_sb, in_=wsq[:])
            wsk_sb = cp.tile([P, 2 * P], F32)
            nc.sync.dma_start(out=wsk_sb, in_=wsk[:])
            bsq_sb = cp.tile([P, 1], F32)
            nc.sync.dma_start(out=bsq_sb, in_=bsq[:])
            bsk_sb = cp.tile([P, 1], F32)
            nc.sync.dma_start(out=bsk_sb, in_=bsk[:])
            ones32 = cp.tile([P, 2], MMDT)
            nc.sync.dma_start(out=ones32, in_=sumw[:])
            zero_b = cp.tile([P, 1], F32)
            nc.vector.memset(zero_b, 0.0)

            # ---------------- speaker rates ----------------
            # psum[p, b] = sum_d Ws[d] * spk[b, d]   (replicated over p)
            rate_sb = cp.tile([P, 2 * BL], F32)   # [:, 0:BL]=rate_q, [:, BL:]=rate_k
            for which, wmat, bias in ((0, wsq_sb, bsq_sb), (1, wsk_sb, bsk_sb)):
                ps = pp.tile([P, 512], F32, tag="ps")
                for kt in range(2):
                    nc.tensor.matmul(
                        ps[:, :BL],
                        wmat[:, kt * P:(kt + 1) * P],
                        spk_sb[:, kt * BL:(kt + 1) * BL],
                        start=(kt == 0),
                        stop=(kt == 1),
                    )
                nc.scalar.activation(
                    rate_sb[:, which * BL:(which + 1) * BL], ps[:, :BL],
                    AF.Sigmoid, bias=bias, scale=1.0,
                )
            # srate[p, b*DT + dt] = rate_b * invdiv[p, dt]
            srq = cp.tile([P, BL, DT], F32)
            srk = cp.tile([P, BL, DT], F32)
            for b in range(BL):
                nc.vector.tensor_scalar_mul(srq[:, b], invd_sb, rate_sb[:, b:b + 1])
                nc.vector.tensor_scalar_mul(
                    srk[:, b], invd_sb, rate_sb[:, BL + b:BL + b + 1])

            # ---------------- per-batch pipeline ----------------
            for b in range(BL):
                kT = bp.tile([P, DT, S], MMDT, tag="kT")     # k^T  [p_j, jt, s]
                v1 = bp.tile([P, DT, S], MMDT, tag="v1")     # v1^T [p_j, jt, s]
                qT = bp.tile([P, DT, T], MMDT, tag="qT")     # q^T  [p_j, jt, t]
                vpp = bp.tile([P, ST, D], MMDT, tag="vpp")   # (v1 @ Wo.T) [p_s, st, e]
                Et = bp.tile([P, ST, T], MMDT, tag="Et")     # exp(scores^T) [p_s, st, t]
                recip = bp.tile([P, TT], F32, tag="recip")

                # ---- k^T = Wk @ (keys^T + posenc): dt-outer, accumulate over dt
                def proj_in(dst_ps, w_dram, x_dram, pos_bc, srate, length, tcol):
                    # one dt-slice: load weight chunk + input chunk (+posenc), 8 MMs
                    for dt in range(DT):
                        wch = wp.tile([P, D], MMDT, tag="w")
                        nc.sync.dma_start(out=wch, in_=w_dram[dt])
                        xch = ip.tile([P, 512], MMDT, tag="x")
                        nc.sync.dma_start(
                            out=xch[:, :length],
                            in_=x_dram[b, dt] if tcol is None
                            else x_dram[b, dt, :, tcol * 512:(tcol + 1) * 512],
                        )
                        if pos_bc is not None:
                            L = length
                            ang = tp.tile([P, 512], F32, tag="ang")
                            # ang = pos * (rate * invdiv) + phase   (>= 0)
                            nc.vector.tensor_scalar(
                                ang[:, :L], pos_bc, srate[:, dt:dt + 1],
                                phs_sb[:, dt:dt + 1], ALU.mult, ALU.add,
                            )
                            # range-reduce to [-pi, pi]: the HW f32->i32 cast
                            # rounds to nearest, so r = ang - 2pi*rint(ang/2pi)
                            yi = tp.tile([P, 512], mybir.dt.int32, tag="yi")
                            nc.scalar.activation(
                                yi[:, :L], ang[:, :L], AF.Copy,
                                bias=0.0, scale=1.0 / (2.0 * PI))
                            nc.vector.scalar_tensor_tensor(
                                ang[:, :L], yi[:, :L], -2.0 * PI, ang[:, :L],
                                ALU.mult, ALU.add)
                            pe = tp.tile([P, 512], F32, tag="pe")
                            nc.scalar.activation(
                                pe[:, :L], ang[:, :L], AF.Sin,
                                bias=zero_b, scale=1.0)
                            nc.gpsimd.tensor_tensor(
                                xch[:, :L], xch[:, :L], pe[:, :L], ALU.add)
                        for jt in range(DT):
                            nc.tensor.matmul(
                                dst_ps[jt][:, :length],
                                wch[:, jt * P:(jt + 1) * P],
                                xch[:, :length],
                                start=(dt == 0),
                                stop=(dt == DT - 1),
                            )

                # k^T
                ps_k = [pp.tile([P, 512], F32, tag="ps", name=f"ps_k{i}") for i in range(DT)]
                proj_in(ps_k, wk, kTi, posk_bc, srk[:, b], S, None)
                for jt in range(DT):
                    nc.vector.tensor_scalar_add(
                        kT[:, jt], ps_k[jt][:, :S], bk_sb[:, jt:jt + 1])

                # v1^T
                ps_v = [pp.tile([P, 512], F32, tag="ps", name=f"ps_v{i}") for i in range(DT)]
                proj_in(ps_v, wv, vTi, None, None, S, None)
                for jt in range(DT):
                    nc.vector.tensor_scalar_add(
                        v1[:, jt], ps_v[jt][:, :S], bv_sb[:, jt:jt + 1])

                # q^T (two 512-column halves)
                for tc in range(2):
                    ps_q = [pp.tile([P, 512], F32, tag="ps", name=f"ps_q{i}") for i in range(DT)]
                    proj_in(ps_q, wq,
                            qTi, posq_bc[:, tc * 512:(tc + 1) * 512],
                            srq[:, b], 512, tc)
                    for jt in range(DT):
                        nc.vector.tensor_scalar_add(
                            qT[:, jt, tc * 512:(tc + 1) * 512], ps_q[jt],
                            bq_sb[:, jt:jt + 1])

                # ---- vpp = v1 @ Wo.T : [p_s, st, e], accumulate over jt
                ps_o = [pp.tile([P, 512], F32, tag="ps", name=f"ps_o{i}") for i in range(2 * ST)]
                for jt in range(DT):
                    wch = wp.tile([P, D], MMDT, tag="w")
                    nc.sync.dma_start(out=wch, in_=wo[jt])
                    for st in range(ST):
                        for ec in range(2):
                            nc.tensor.matmul(
                                ps_o[st * 2 + ec],
                                v1[:, jt, st * P:(st + 1) * P],
                                wch[:, ec * 512:(ec + 1) * 512],
                                start=(jt == 0),
                                stop=(jt == DT - 1),
                            )
                for st in range(ST):
                    for ec in range(2):
                        nc.scalar.activation(
                            vpp[:, st, ec * 512:(ec + 1) * 512], ps_o[st * 2 + ec],
                            AF.Copy, bias=0.0, scale=1.0)

                # ---- E = exp(scores^T / sqrt(d)) : [p_s, st, t], acc over jt
                ps_s = [pp.tile([P, 512], F32, tag="ps", name=f"ps_s{i}") for i in range(2 * ST)]
                for jt in range(DT):
                    for st in range(ST):
                        for tc in range(2):
                            nc.tensor.matmul(
                                ps_s[st * 2 + tc],
                                kT[:, jt, st * P:(st + 1) * P],
                                qT[:, jt, tc * 512:(tc + 1) * 512],
                                start=(jt == 0),
                                stop=(jt == DT - 1),
                            )
                for st in range(ST):
                    for tc in range(2):
                        nc.scalar.activation(
                            Et[:, st, tc * 512:(tc + 1) * 512], ps_s[st * 2 + tc],
                            AF.Exp, bias=zero_b, scale=1.0 / SQRT_D)

                # ---- recip[t] = 1 / (sqrt(d) * sum_s E[s,t])
                for tt in range(TT):
                    ps1 = pp.tile([P, 512], F32, tag="ps")
                    for st in range(ST):
                        nc.tensor.matmul(
                            ps1[:, :2],
                            Et[:, st, tt * P:(tt + 1) * P],
                            ones32,
                            start=(st == 0),
                            stop=(st == ST - 1),
                        )
                    nc.vector.reciprocal(recip[:, tt:tt + 1], ps1[:, 0:1])

                # ---- out[t, e] = recip[t] * sum_s E[s,t] vpp[s,e] + bo
                for th in range(2):
                    ps_f = [pp.tile([P, 512], F32, tag="ps", name=f"ps_f{i}") for i in range(8)]
                    for st in range(ST):
                        for ti in range(4):
                            for ec in range(2):
                                nc.tensor.matmul(
                                    ps_f[ti * 2 + ec],
                                    Et[:, st, (th * 4 + ti) * P:(th * 4 + ti + 1) * P],
                                    vpp[:, st, ec * 512:(ec + 1) * 512],
                                    start=(st == 0),
                                    stop=(st == ST - 1),
                                )
                    for ti in range(4):
                        tt = th * 4 + ti
                        osb = op.tile([P, D], F32, tag="osb")
                        for ec in range(2):
                            nc.vector.scalar_tensor_tensor(
                                osb[:, ec * 512:(ec + 1) * 512],
                                ps_f[ti * 2 + ec], recip[:, tt:tt + 1],
                                bo_bc[:, ec * 512:(ec + 1) * 512],
                                ALU.mult, ALU.add)
                        nc.sync.dma_start(out=out[b, tt], in_=osb)
    return nc


def marshal_inputs(query, keys, values, speaker_embedding, Wsq, bsq, Wsk, bsk,
                   Wq, bq, Wk, bk, Wv, bv, Wo, bo, current_mel_pos):
    f = lambda x: np.ascontiguousarray(np.asarray(x, dtype=np.float32))
    query, keys, values = f(query), f(keys), f(values)
    spk = f(speaker_embedding)
    Wsq, Wsk = f(Wsq), f(Wsk)
    Wq, Wk, Wv, Wo = f(Wq), f(Wk), f(Wv), f(Wo)
    bq, bk, bv, bo = f(bq), f(bk), f(bv), f(bo)
    bsq, bsk = f(bsq), f(bsk)
    mel0 = int(np.asarray(current_mel_pos).item())

    dvec = np.arange(D, dtype=np.float32)
    invdiv = (10000.0 ** (-dvec / D)).astype(np.float32)
    phase = np.where(dvec.astype(np.int64) % 2 == 0, 0.0, np.pi / 2).astype(np.float32)
    phase_pi = (phase + np.pi).astype(np.float32)

    col = lambda v: np.ascontiguousarray(v.reshape(DT, P).T)
    shared = {
        "wq": np.ascontiguousarray(Wq.T.reshape(DT, P, D)),
        "wk": np.ascontiguousarray(Wk.T.reshape(DT, P, D)),
        "wv": np.ascontiguousarray(Wv.T.reshape(DT, P, D)),
        "wo": np.ascontiguousarray(Wo.T.reshape(DT, P, D)),
        "posq": (np.arange(T, dtype=np.float32) + mel0),
        "posk": np.arange(S, dtype=np.float32),
        "invd": col(invdiv),
        "phs": col(phase),
        "bqc": col(bq),
        "bkc": col(bk),
        "bvc": col(bv),
        "boc": bo,
        "wsq": np.ascontiguousarray(
            np.repeat(Wsq.reshape(2, P, 1), P, axis=2).transpose(1, 0, 2).reshape(P, 2 * P)),
        "wsk": np.ascontiguousarray(
            np.repeat(Wsk.reshape(2, P, 1), P, axis=2).transpose(1, 0, 2).reshape(P, 2 * P)),
        "bsq": np.full((P, 1), bsq.reshape(-1)[0], dtype=np.float32),
        "sumw": np.full((P, 2), SQRT_D, dtype=np.float32),
        "bsk": np.full((P, 1), bsk.reshape(-1)[0], dtype=np.float32),
    }
    tr = lambda x, L: np.ascontiguousarray(
        x.reshape(BL, L, DT, P).transpose(0, 2, 3, 1))
    in_maps = []
    for c in range(NCORES):
        sl = slice(c * BL, (c + 1) * BL)
        m = dict(shared)
        m["qTi"] = tr(query[sl], T)
        m["kTi"] = tr(keys[sl], S)
        m["vTi"] = tr(values[sl], S)
        m["spk"] = np.ascontiguousarray(
            spk[sl].T.reshape(2, P, BL).transpose(1, 0, 2).reshape(P, 2 * BL))
        in_maps.append(m)
    return in_maps


def run_device(in_maps, trace=False, **kw):
    nc = build_nc()
    if not nc.is_finalized():
        nc.finalize()
    res = run_bass_kernel_spmd(nc, in_maps, core_ids=list(range(NCORES)),
                               trace=trace, **kw)
    outs = [np.asarray(r["out"], dtype=np.float32).reshape(BL, T, D)
            for r in res.results]
    return np.concatenate(outs, axis=0), res


def kernel(**inputs) -> np.ndarray:
    in_maps = marshal_inputs(**inputs)
    out, _ = run_device(in_maps)
    return out


# revision 17
# speedup vs baseline: 1.0989x; 1.0989x over previous
"""Trainium2 Bass kernel for speaker-rate positional-encoding attention.

Math (per batch b):
  rate_q = sigmoid(spk @ Wsq.T + bsq);  rate_k = sigmoid(spk @ Wsk.T + bsk)
  pe(x)[l,d] = sin(rate * pos[l] * 10000^(-d/D) + phase[d]),  phase = 0/pi/2 (even/odd d)
  q = (query + pe_q) @ Wq.T + bq ; k = (keys + pe_k) @ Wk.T + bk
  v1 = values @ Wv.T + bv ; vpp = v1 @ Wo.T
  E = exp((k q^T)/sqrt(D)) ; out[t] = (sum_s E[s,t] vpp[s,:]) / (sqrt(D)*sum_s E[s,t]) + bo

Implementation notes:
 - 8 NeuronCores, data-parallel over batch (2 per core), no collectives.
 - All activations live feature-on-partition ("transposed"); the host
   pre-transposes inputs and weights so the device never transposes.
 - Matmuls run in float32r (full-rate fp32 on the PE for N>=256).
 - Softmax skips max-subtraction (scores bounded ~ +-15) and defers
   normalization to a per-partition scalar multiply after E @ vpp.
 - Sin LUT domain is [-pi, pi]: angles are range-reduced with
   ((x + pi) mod 2pi) - pi  (all angles >= 0 so floor-mod is safe).
"""

import sys

for _p in ("/opt/trn_rl_repo",):
    if _p not in sys.path:
        sys.path.insert(0, _p)

import numpy as np

import concourse.bass as bass
from concourse import bacc
import concourse.mybir as mybir
import concourse.tile as tile
from concourse.bass_utils import run_bass_kernel_spmd

B, T, S, D, SPK = 16, 1024, 512, 1024, 256
NCORES = 8
BL = B // NCORES          # batches per core
P = 128
DT = D // P               # 8 d-tiles (also j-tiles / e-tiles)
ST = S // P               # 4 s-tiles
TT = T // P               # 8 t-tiles
F32 = mybir.dt.float32
MMDT = mybir.dt.float32r  # matmul compute dtype
SQRT_D = float(np.sqrt(D))
PI = float(np.pi)

AF = mybir.ActivationFunctionType
ALU = mybir.AluOpType


def build_nc():
    nc = bacc.Bacc()
    dp = nc.declare_dram_parameter
    qTi = dp("qTi", [BL, DT, P, T], MMDT, isOutput=False)   # query^T  [b,dt,p_d,t]
    kTi = dp("kTi", [BL, DT, P, S], MMDT, isOutput=False)   # keys^T
    vTi = dp("vTi", [BL, DT, P, S], MMDT, isOutput=False)   # values^T
    wq = dp("wq", [DT, P, D], MMDT, isOutput=False)         # Wq.T [dt,p_d,j]
    wk = dp("wk", [DT, P, D], MMDT, isOutput=False)
    wv = dp("wv", [DT, P, D], MMDT, isOutput=False)
    wo = dp("wo", [DT, P, D], MMDT, isOutput=False)         # Wo.T [jt,p_j,e]
    posq = dp("posq", [T], F32, isOutput=False)             # current_mel_pos + t
    posk = dp("posk", [S], F32, isOutput=False)
    invd = dp("invd", [P, DT], F32, isOutput=False)         # 10000^(-d/D)
    phs = dp("phs", [P, DT], F32, isOutput=False)           # 0 / pi/2
    bqc = dp("bqc", [P, DT], F32, isOutput=False)
    bkc = dp("bkc", [P, DT], F32, isOutput=False)
    bvc = dp("bvc", [P, DT], F32, isOutput=False)
    boc = dp("boc", [D], F32, isOutput=False)
    spk = dp("spk", [P, 2 * BL], F32, isOutput=False)      # spk^T  [p, kt*BL+b]
    wsq = dp("wsq", [P, 2 * P], F32, isOutput=False)       # Wsq col-replicated
    wsk = dp("wsk", [P, 2 * P], F32, isOutput=False)
    bsq = dp("bsq", [P, 1], F32, isOutput=False)
    sumw = dp("sumw", [P, 2], MMDT, isOutput=False)   # sqrt(D) ones column
    bsk = dp("bsk", [P, 1], F32, isOutput=False)
    out = dp("out", [BL, TT, P, D], F32, isOutput=True)

    def bcast(ap, n=P):
        # replicate a DRAM vector across n partitions (DMA partition-step 0)
        return bass.AP(tensor=ap.tensor, offset=ap.offset, ap=[[0, n], *ap.ap])

    with tile.TileContext(nc) as tc:
        with (
            tc.tile_pool(name="consts", bufs=1) as cp,
            tc.tile_pool(name="wch", bufs=6) as wp,       # weight chunks [P, D]
            tc.tile_pool(name="inch", bufs=6) as ip,      # input act chunks
            tc.tile_pool(name="tmp", bufs=6) as tp,       # posenc temps
            tc.tile_pool(name="big", bufs=1) as bp,       # long-lived per-batch acts
            tc.tile_pool(name="outp", bufs=4) as op,
            tc.tile_pool(name="psum", bufs=8, space="PSUM") as pp,
        ):
            # ---------------- constants ----------------
            posq_bc = cp.tile([P, T], F32)
            nc.gpsimd.dma_start(out=posq_bc, in_=bcast(posq[:]))
            posk_bc = cp.tile([P, S], F32)
            nc.gpsimd.dma_start(out=posk_bc, in_=bcast(posk[:]))
            bo_bc = cp.tile([P, D], F32)
            nc.gpsimd.dma_start(out=bo_bc, in_=bcast(boc[:]))
            invd_sb = cp.tile([P, DT], F32)
            nc.sync.dma_start(out=invd_sb, in_=invd[:])
            phs_sb = cp.tile([P, DT], F32)
            nc.sync.dma_start(out=phs_sb, in_=phs[:])
            bq_sb = cp.tile([P, DT], F32)
            nc.sync.dma_start(out=bq_sb, in_=bqc[:])
            bk_sb = cp.tile([P, DT], F32)
            nc.sync.dma_start(out=bk_sb, in_=bkc[:])
            bv_sb = cp.tile([P, DT], F32)
            nc.sync.dma_start(out=bv_sb, in_=bvc[:])
            spk_sb = cp.tile([P, 2 * BL], F32)
            nc.sync.dma_start(out=spk_sb, in_=spk[:])
            wsq_sb = cp.tile([P, 2 * P], F32)
            nc.sync.dma_start(out=wsq_sb, in_=wsq[:])
            wsk_sb = cp.tile([P, 2 * P], F32)
            nc.sync.dma_start(out=wsk_sb, in_=wsk[:])
            bsq_sb = cp.tile([P, 1], F32)
            nc.sync.dma_start(out=bsq_sb, in_=bsq[:])
            bsk_sb = cp.tile([P, 1], F32)
            nc.sync.dma_start(out=bsk_sb, in_=bsk[:])
            ones32 = cp.tile([P, 2], MMDT)
            nc.sync.dma_start(out=ones32, in_=sumw[:])
            zero_b = cp.tile([P, 1], F32)
            nc.vector.memset(zero_b, 0.0)

            # ---------------- speaker rates ----------------
            # psum[p, b] = sum_d Ws[d] * spk[b, d]   (replicated over p)
            rate_sb = cp.tile([P, 2 * BL], F32)   # [:, 0:BL]=rate_q, [:, BL:]=rate_k
            for which, wmat, bias in ((0, wsq_sb, bsq_sb), (1, wsk_sb, bsk_sb)):
                ps = pp.tile([P, 512], F32, tag="ps")
                for kt in range(2):
                    nc.tensor.matmul(
                        ps[:, :BL],
                        wmat[:, kt * P:(kt + 1) * P],
                        spk_sb[:, kt * BL:(kt + 1) * BL],
                        start=(kt == 0),
                        stop=(kt == 1),
                    )
                nc.scalar.activation(
                    rate_sb[:, which * BL:(which + 1) * BL], ps[:, :BL],
                    AF.Sigmoid, bias=bias, scale=1.0,
                )
            # srate[p, b*DT + dt] = rate_b * invdiv[p, dt]
            srq = cp.tile([P, BL, DT], F32)
            srk = cp.tile([P, BL, DT], F32)
            for b in range(BL):
                nc.vector.tensor_scalar_mul(srq[:, b], invd_sb, rate_sb[:, b:b + 1])
                nc.vector.tensor_scalar_mul(
                    srk[:, b], invd_sb, rate_sb[:, BL + b:BL + b + 1])

            # ---------------- per-batch pipeline ----------------
            for b in range(BL):
                kT = bp.tile([P, DT, S], MMDT, tag="kT")     # k^T  [p_j, jt, s]
                v1 = bp.tile([P, DT, S], MMDT, tag="v1")     # v1^T [p_j, jt, s]
                qT = bp.tile([P, DT, T], MMDT, tag="qT")     # q^T  [p_j, jt, t]
                vpp = bp.tile([P, ST, D], MMDT, tag="vpp")   # (v1 @ Wo.T) [p_s, st, e]
                Et = bp.tile([P, ST, T], MMDT, tag="Et")     # exp(scores^T) [p_s, st, t]
                recip = bp.tile([P, TT], F32, tag="recip")

                # ---- k^T = Wk @ (keys^T + posenc): dt-outer, accumulate over dt
                def proj_in(dst_ps, w_dram, x_dram, pos_bc, srate, length, tcol):
                    # one dt-slice: load weight chunk + input chunk (+posenc), 8 MMs
                    for dt in range(DT):
                        wch = wp.tile([P, D], MMDT, tag="w")
                        nc.sync.dma_start(out=wch, in_=w_dram[dt])
                        xch = ip.tile([P, 512], MMDT, tag="x")
                        nc.sync.dma_start(
                            out=xch[:, :length],
                            in_=x_dram[b, dt] if tcol is None
                            else x_dram[b, dt, :, tcol * 512:(tcol + 1) * 512],
                        )
                        if pos_bc is not None:
                            L = length
                            ang = tp.tile([P, 512], F32, tag="ang")
                            # ang = pos * (rate * invdiv) + phase   (>= 0)
                            nc.vector.tensor_scalar(
                                ang[:, :L], pos_bc, srate[:, dt:dt + 1],
                                phs_sb[:, dt:dt + 1], ALU.mult, ALU.add,
                            )
                            # range-reduce to [-pi, pi]: the HW f32->i32 cast
                            # rounds to nearest, so r = ang - 2pi*rint(ang/2pi)
                            yi = tp.tile([P, 512], mybir.dt.int32, tag="yi")
                            nc.scalar.activation(
                                yi[:, :L], ang[:, :L], AF.Copy,
                                bias=0.0, scale=1.0 / (2.0 * PI))
                            nc.vector.scalar_tensor_tensor(
                                ang[:, :L], yi[:, :L], -2.0 * PI, ang[:, :L],
                                ALU.mult, ALU.add)
                            pe = tp.tile([P, 512], F32, tag="pe")
                            nc.scalar.activation(
                                pe[:, :L], ang[:, :L], AF.Sin,
                                bias=zero_b, scale=1.0)
                            nc.vector.tensor_add(
                                xch[:, :L], xch[:, :L], pe[:, :L])
                        for jt in range(DT):
                            nc.tensor.matmul(
                                dst_ps[jt][:, :length],
                                wch[:, jt * P:(jt + 1) * P],
                                xch[:, :length],
                                start=(dt == 0),
                                stop=(dt == DT - 1),
                            )

                # v1^T first: no posenc dependency, so the PE starts on
                # DMA-only chunks while the keys posenc pipeline fills
                ps_v = [pp.tile([P, 512], F32, tag="ps", name=f"ps_v{i}") for i in range(DT)]
                proj_in(ps_v, wv, vTi, None, None, S, None)
                for jt in range(DT):
                    nc.scalar.activation(
                        v1[:, jt], ps_v[jt][:, :S], AF.Identity,
                        bias=bv_sb[:, jt:jt + 1], scale=1.0)

                # k^T
                ps_k = [pp.tile([P, 512], F32, tag="ps", name=f"ps_k{i}") for i in range(DT)]
                proj_in(ps_k, wk, kTi, posk_bc, srk[:, b], S, None)
                for jt in range(DT):
                    nc.scalar.activation(
                        kT[:, jt], ps_k[jt][:, :S], AF.Identity,
                        bias=bk_sb[:, jt:jt + 1], scale=1.0)

                # q^T (two 512-column halves)
                for tc in range(2):
                    ps_q = [pp.tile([P, 512], F32, tag="ps", name=f"ps_q{i}") for i in range(DT)]
                    proj_in(ps_q, wq,
                            qTi, posq_bc[:, tc * 512:(tc + 1) * 512],
                            srq[:, b], 512, tc)
                    for jt in range(DT):
                        nc.scalar.activation(
                            qT[:, jt, tc * 512:(tc + 1) * 512], ps_q[jt],
                            AF.Identity, bias=bq_sb[:, jt:jt + 1], scale=1.0)

                # ---- vpp = v1 @ Wo.T : [p_s, st, e], accumulate over jt
                ps_o = [pp.tile([P, 512], F32, tag="ps", name=f"ps_o{i}") for i in range(2 * ST)]
                for jt in range(DT):
                    wch = wp.tile([P, D], MMDT, tag="w")
                    nc.sync.dma_start(out=wch, in_=wo[jt])
                    for st in range(ST):
                        for ec in range(2):
                            nc.tensor.matmul(
                                ps_o[st * 2 + ec],
                                v1[:, jt, st * P:(st + 1) * P],
                                wch[:, ec * 512:(ec + 1) * 512],
                                start=(jt == 0),
                                stop=(jt == DT - 1),
                            )
                for st in range(ST):
                    for ec in range(2):
                        nc.scalar.activation(
                            vpp[:, st, ec * 512:(ec + 1) * 512], ps_o[st * 2 + ec],
                            AF.Copy, bias=0.0, scale=1.0)

                # ---- E = exp(scores^T / sqrt(d)) : [p_s, st, t], acc over jt
                ps_s = [pp.tile([P, 512], F32, tag="ps", name=f"ps_s{i}") for i in range(2 * ST)]
                for jt in range(DT):
                    for st in range(ST):
                        for tc in range(2):
                            nc.tensor.matmul(
                                ps_s[st * 2 + tc],
                                kT[:, jt, st * P:(st + 1) * P],
                                qT[:, jt, tc * 512:(tc + 1) * 512],
                                start=(jt == 0),
                                stop=(jt == DT - 1),
                            )
                for st in range(ST):
                    for tc in range(2):
                        nc.scalar.activation(
                            Et[:, st, tc * 512:(tc + 1) * 512], ps_s[st * 2 + tc],
                            AF.Exp, bias=zero_b, scale=1.0 / SQRT_D)

                # ---- recip[t] = 1 / (sqrt(d) * sum_s E[s,t])
                for tt in range(TT):
                    ps1 = pp.tile([P, 512], F32, tag="ps")
                    for st in range(ST):
                        nc.tensor.matmul(
                            ps1[:, :2],
                            Et[:, st, tt * P:(tt + 1) * P],
                            ones32,
                            start=(st == 0),
                            stop=(st == ST - 1),
                        )
                    nc.vector.reciprocal(recip[:, tt:tt + 1], ps1[:, 0:1])

                # ---- out[t, e] = recip[t] * sum_s E[s,t] vpp[s,e] + bo
                for th in range(2):
                    ps_f = [pp.tile([P, 512], F32, tag="ps", name=f"ps_f{i}") for i in range(8)]
                    for st in range(ST):
                        for ti in range(4):
                            for ec in range(2):
                                nc.tensor.matmul(
                                    ps_f[ti * 2 + ec],
                                    Et[:, st, (th * 4 + ti) * P:(th * 4 + ti + 1) * P],
                                    vpp[:, st, ec * 512:(ec + 1) * 512],
                                    start=(st == 0),
                                    stop=(st == ST - 1),
                                )
                    for ti in range(4):
                        tt = th * 4 + ti
                        osb = op.tile([P, D], F32, tag="osb")
                        for ec in range(2):
                            nc.vector.scalar_tensor_tensor(
                                osb[:, ec * 512:(ec + 1) * 512],
                                ps_f[ti * 2 + ec], recip[:, tt:tt + 1],
                                bo_bc[:, ec * 512:(ec + 1) * 512],
                                ALU.mult, ALU.add)
                        nc.sync.dma_start(out=out[b, tt], in_=osb)
    return nc


def marshal_inputs(query, keys, values, speaker_embedding, Wsq, bsq, Wsk, bsk,
                   Wq, bq, Wk, bk, Wv, bv, Wo, bo, current_mel_pos):
    f = lambda x: np.ascontiguousarray(np.asarray(x, dtype=np.float32))
    query, keys, values = f(query), f(keys), f(values)
    spk = f(speaker_embedding)
    Wsq, Wsk = f(Wsq), f(Wsk)
    Wq, Wk, Wv, Wo = f(Wq), f(Wk), f(Wv), f(Wo)
    bq, bk, bv, bo = f(bq), f(bk), f(bv), f(bo)
    bsq, bsk = f(bsq), f(bsk)
    mel0 = int(np.asarray(current_mel_pos).item())

    dvec = np.arange(D, dtype=np.float32)
    invdiv = (10000.0 ** (-dvec / D)).astype(np.float32)
    phase = np.where(dvec.astype(np.int64) % 2 == 0, 0.0, np.pi / 2).astype(np.float32)
    phase_pi = (phase + np.pi).astype(np.float32)

    col = lambda v: np.ascontiguousarray(v.reshape(DT, P).T)
    shared = {
        "wq": np.ascontiguousarray(Wq.T.reshape(DT, P, D)),
        "wk": np.ascontiguousarray(Wk.T.reshape(DT, P, D)),
        "wv": np.ascontiguousarray(Wv.T.reshape(DT, P, D)),
        "wo": np.ascontiguousarray(Wo.T.reshape(DT, P, D)),
        "posq": (np.arange(T, dtype=np.float32) + mel0),
        "posk": np.arange(S, dtype=np.float32),
        "invd": col(invdiv),
        "phs": col(phase),
        "bqc": col(bq),
        "bkc": col(bk),
        "bvc": col(bv),
        "boc": bo,
        "wsq": np.ascontiguousarray(
            np.repeat(Wsq.reshape(2, P, 1), P, axis=2).transpose(1, 0, 2).reshape(P, 2 * P)),
        "wsk": np.ascontiguousarray(
            np.repeat(Wsk.reshape(2, P, 1), P, axis=2).transpose(1, 0, 2).reshape(P, 2 * P)),
        "bsq": np.full((P, 1), bsq.reshape(-1)[0], dtype=np.float32),
        "sumw": np.full((P, 2), SQRT_D, dtype=np.float32),
        "bsk": np.full((P, 1), bsk.reshape(-1)[0], dtype=np.float32),
    }
    tr = lambda x, L: np.ascontiguousarray(
        x.reshape(BL, L, DT, P).transpose(0, 2, 3, 1))
    in_maps = []
    for c in range(NCORES):
        sl = slice(c * BL, (c + 1) * BL)
        m = dict(shared)
        m["qTi"] = tr(query[sl], T)
        m["kTi"] = tr(keys[sl], S)
        m["vTi"] = tr(values[sl], S)
        m["spk"] = np.ascontiguousarray(
            spk[sl].T.reshape(2, P, BL).transpose(1, 0, 2).reshape(P, 2 * BL))
        in_maps.append(m)
    return in_maps


def run_device(in_maps, trace=False, **kw):
    nc = build_nc()
    if not nc.is_finalized():
        nc.finalize()
    res = run_bass_kernel_spmd(nc, in_maps, core_ids=list(range(NCORES)),
                               trace=trace, **kw)
    outs = [np.asarray(r["out"], dtype=np.float32).reshape(BL, T, D)
            for r in res.results]
    return np.concatenate(outs, axis=0), res


def kernel(**inputs) -> np.ndarray:
    in_maps = marshal_inputs(**inputs)
    out, _ = run_device(in_maps)
    return out
